# revision 1
# baseline (speedup 1.0000x reference)
"""Trainium2 Bass/Tile kernel for EntropyRecyclingLanguageNet (vq_codebook).

Computes, for x[B,D]:
    pw    = softmax(x @ attn_w + attn_b)          # [B,P]
    rec   = pw @ pattern_dict                      # [B,D]
    par   = rec @ self_w + self_b - rec            # [B,D]
    out   = (rec * sigmoid(||par||)) @ out_w + out_b   # [B,V]

Sharding: tensor-parallel over the vocab dim (V=32000 -> 4000 per core);
every core runs the full small stage for all B rows (cheap), and the
dominant cost -- the [8192, 4000] output slice -- is spread across the
8 cores.  Host gathers with a concat along axis 1 (+ f32 upcast).

v4 design notes (driven by perfetto traces + the engine cost models):
  * fp16 everywhere the 2e-2 rel-err gate allows: inputs, fused factors,
    and the OUTPUT (halves the dominant HBM write).  Measured rel err
    stays ~6e-4.
  * weight-only fusions are host-side preprocessing (like the baseline's
    self_w - I):  m3aug = [[pdict@(self_w-I) | 1 | 2*pdict@(self_w-I)@self_b],
    [0...]]  [P+1, D+2] and m2aug = [[pdict @ out_w], [out_b]]  [P+1, VS].
  * small stage runs transposed: logitsT[P,W] = attn_w.T @ xT-block, Exp
    with attn_b folded gives expwT (unnormalized softmax numerators).
  * ONE K=65 matmul per tile against m3aug yields, per row b:
    cols 0:D   = d_b * (rec_true@(self_w-I))      (paradox, scaled by denom)
    col  D     = d_b                               (softmax denominator)
    col  D+1   = 2*d_b*(par_nb . self_b)           (self_b cross term)
    so ||par_true||^2 * d^2 = q_nb + d*col_{D+1} + d^2*||self_b||^2 --
    no PE transpose, no ones-matmul, no ACT-accum for the denominator
    (a 2-wide DVE copy extracts cols D:D+2).
  * gate tail is table-switch-free on ACT: sqrt via int-shift seed + one
    Newton step on DVE, sigmoid via the already-loaded Exp table
    (scl = sig/d = 1/((1+e^-m)*d), rps = 1/scl).  ONE ACT table load for
    the whole kernel (switches cost ~2.7us each).
  * phase B uses ewT directly as the stationary operand: row P is
    overwritten per group with rps (thin PE transposes + one [1,512] DVE
    row copy per block), so out = scl*(expw @ m2aug + rps*out_b) with the
    per-row scl applied FREE inside the PSUM->SBUF drain (scaled copy).
  * drains are 2-PSUM-banks wide ([128,1024] per instruction, pso bufs=3
    so ACT and DVE drains run concurrently) to amortize fixed overheads.
  * groups are software-pipelined (phase A of g+1 emitted before phase B
    of g) so the Tensor engine -- the critical engine at ~265us busy --
    almost never idles.
"""

import numpy as np

import concourse.bass as bass
import concourse.mybir as mybir
import concourse.tile as tile
from concourse import bacc
from concourse.bass_utils import run_bass_kernel_spmd

B, D, P, V = 8192, 128, 64, 32000
NCORES = 8
VS = V // NCORES        # vocab cols per core (4000)
BT = 128                # batch tile (partition dim)
NBT = B // BT           # 64 batch tiles
G = 8                   # batch tiles per group
W = 512                 # small-stage block width (4 batch tiles)
NBLK = W // BT          # batch tiles per block (4)
DC = D + 2              # par-matmul cols: paradox | denom | self_b-dot
F32 = mybir.dt.float32
F16 = mybir.dt.float16
U32 = mybir.dt.uint32
AF = mybir.ActivationFunctionType
ALU = mybir.AluOpType
SQRT_MAGIC = 0x1FBD1DF5  # float-bit sqrt seed: sqrt(x) ~ bitcast((i>>1)+C)

# vocab drain pairs: two 512-wide matmuls share a 2-bank PSUM tile
VPAIRS = [(512, 512), (512, 512), (512, 512), (512, 416)]

_cache = {}


def _build():
    nc = bacc.Bacc(
        "TRN2",
        target_bir_lowering=False,
        debug=False,
        num_devices=NCORES,
    )

    d_xT = nc.dram_tensor("xT", [D, B], F16, kind="ExternalInput").ap()
    d_attn_w = nc.dram_tensor("attn_w", [D, P], F16, kind="ExternalInput").ap()
    d_attn_b = nc.dram_tensor("attn_b", [P, 1], F32, kind="ExternalInput").ap()
    d_ident = nc.dram_tensor("ident", [128, 128], F16, kind="ExternalInput").ap()
    d_sb2 = nc.dram_tensor("sb2", [BT, 1], F32, kind="ExternalInput").ap()
    d_m3aug = nc.dram_tensor("m3aug", [P + 1, DC], F16, kind="ExternalInput").ap()
    d_m2aug = nc.dram_tensor("m2aug", [P + 1, VS], F16, kind="ExternalInput").ap()
    d_out = nc.dram_tensor("out", [B, VS], F16, kind="ExternalOutput").ap()

    with tile.TileContext(nc) as tc:
        with (
            tc.tile_pool(name="consts", bufs=1) as cpool,
            tc.tile_pool(name="wide", bufs=6) as wpool,
            tc.tile_pool(name="grp", bufs=2) as gpool,
            tc.tile_pool(name="small", bufs=3) as spool,
            tc.tile_pool(name="stage", bufs=6) as stpool,
            tc.tile_pool(name="pss", bufs=2, space="PSUM") as pss,
            tc.tile_pool(name="pso", bufs=3, space="PSUM") as pso,
        ):
            # ---- resident constants -------------------------------------
            # order: group 0's needs first (attn_w/b + xT chunk 0), then
            # the factors, then the remaining xT chunks (group g uses
            # exactly chunk g)
            attn_w = cpool.tile([D, P], F16)
            nc.sync.dma_start(attn_w[:], d_attn_w[:])
            attn_b = cpool.tile([P, 1], F32)
            nc.sync.dma_start(attn_b[:], d_attn_b[:])
            xT = cpool.tile([D, B], F16)
            nc.sync.dma_start(xT[:, 0:B // 8], d_xT[:, 0:B // 8])
            m3aug = cpool.tile([P + 1, DC], F16)
            nc.sync.dma_start(m3aug[:], d_m3aug[:])
            sb2 = cpool.tile([BT, 1], F32)
            nc.sync.dma_start(sb2[:], d_sb2[:])
            ident = cpool.tile([128, 128], F16)
            nc.sync.dma_start(ident[:], d_ident[:])
            m2aug = cpool.tile([P + 1, VS], F16)
            nc.sync.dma_start(m2aug[:], d_m2aug[:])
            for c in range(1, 8):
                nc.sync.dma_start(
                    xT[:, c * (B // 8):(c + 1) * (B // 8)],
                    d_xT[:, c * (B // 8):(c + 1) * (B // 8)],
                )

            # ---- group state --------------------------------------------
            ewT_blocks = {}   # g -> [block0, block1]
            scl_tiles = {}    # g -> [BT, G] f32 per-row output scale

            def phase_a_block(g, blk, qall, dcall):
                i0 = g * G + blk * NBLK
                c0 = i0 * BT
                ps_logT = pss.tile([P, W], F32, tag="s", name=f"ps_logT_{i0}")
                nc.tensor.matmul(
                    ps_logT[:], attn_w[:], xT[:, c0:c0 + W],
                    start=True, stop=True,
                )
                # rows 0..P-1: expwT = exp(logitsT + attn_b)
                # row P: zeroed now (m3aug row P is 0), rps = d/sig later
                ewT = wpool.tile([P + 1, W], F16, tag="ewT", name=f"ewT_{i0}")
                nc.gpsimd.memset(ewT[P:P + 1, :], 0.0)
                nc.scalar.activation(ewT[0:P, :], ps_logT[:], AF.Exp, bias=attn_b[:])
                ewT_blocks[g].append(ewT)

                for t in range(NBLK):
                    tg = blk * NBLK + t
                    sl = slice(t * BT, (t + 1) * BT)
                    # [paradox*d | d | 2*d*(par.self_b)] in one matmul
                    ps_par = pss.tile([BT, DC], F32, tag="s", name=f"ps_par_{g}_{tg}")
                    nc.tensor.matmul(ps_par[:], ewT[:, sl], m3aug[:], start=True, stop=True)
                    sq = spool.tile([BT, D], F32, tag="sq", name=f"sq_{g}_{tg}")
                    nc.scalar.activation(
                        sq[:], ps_par[:, 0:D], AF.Square,
                        accum_out=qall[:, tg:tg + 1],
                    )
                    nc.vector.tensor_copy(
                        dcall[:, 2 * tg:2 * tg + 2], ps_par[:, D:DC]
                    )

            def tail_scalar(g, qall, dcall):
                # q = q_nb + d*c1 + d^2*||self_b||^2   (c1 holds the 2x)
                # scl = sig(|par|)/d = 1/((1+e^-m)*d),  rps = 1/scl
                # sqrt via int-shift seed + one Newton step (no Sqrt table)
                # high_priority: the serial chain must not interleave with
                # the 1.2us phase-B drains on the DVE, or its latency blows
                # up to ~15us and stalls the PE at the rps transposes
                dd = dcall[:, 0:2 * G:2]
                c1 = dcall[:, 1:2 * G:2]
                with tc.high_priority(offset=120):
                    # off the qall-critical-path (need dcall only)
                    rden = gpool.tile([BT, G], F32, tag="rden", name=f"rden_{g}")
                    nc.vector.reciprocal(rden[:], dd)
                    t1 = gpool.tile([BT, G], F32, tag="t1", name=f"t1_{g}")
                    nc.vector.tensor_mul(t1[:], c1, dd)
                    t4 = gpool.tile([BT, G], F32, tag="t4", name=f"t4_{g}")
                    nc.vector.scalar_tensor_tensor(
                        t4[:], dd, sb2[:], dd, ALU.mult, ALU.mult
                    )
                    # qall-critical chain
                    q1 = gpool.tile([BT, G], F32, tag="q1", name=f"q1_{g}")
                    nc.vector.tensor_add(q1[:], qall[:], t1[:])
                    q2 = gpool.tile([BT, G], F32, tag="q2", name=f"q2_{g}")
                    nc.vector.tensor_add(q2[:], q1[:], t4[:])
                    y0 = gpool.tile([BT, G], F32, tag="y0", name=f"y0_{g}")
                    nc.vector.tensor_scalar(
                        y0[:].bitcast(U32), q2[:].bitcast(U32),
                        1, None, ALU.logical_shift_right,
                    )
                    nc.vector.tensor_scalar(
                        y0[:].bitcast(U32), y0[:].bitcast(U32),
                        SQRT_MAGIC, None, ALU.add,
                    )
                    ry0 = gpool.tile([BT, G], F32, tag="ry0", name=f"ry0_{g}")
                    nc.vector.reciprocal(ry0[:], y0[:])
                    qy = gpool.tile([BT, G], F32, tag="qy", name=f"qy_{g}")
                    nc.vector.tensor_mul(qy[:], q2[:], ry0[:])
                    s2 = gpool.tile([BT, G], F32, tag="s2", name=f"s2_{g}")
                    nc.vector.tensor_add(s2[:], qy[:], y0[:])   # 2*sqrt(q)
                    mg = gpool.tile([BT, G], F32, tag="mg", name=f"mg_{g}")
                    nc.vector.tensor_mul(mg[:], s2[:], rden[:])  # 2*|par|
                    em = gpool.tile([BT, G], F32, tag="em", name=f"em_{g}")
                    nc.scalar.activation(em[:], mg[:], AF.Exp, scale=-0.5)
                    rps = gpool.tile([BT, G], F32, tag="rps", name=f"rps_{g}")
                    nc.vector.scalar_tensor_tensor(
                        rps[:], em[:], 1.0, dd, ALU.add, ALU.mult  # (em+1)*d
                    )
                    rps16 = gpool.tile([BT, G], F16, tag="rps16", name=f"rps16_{g}")
                    nc.vector.tensor_copy(rps16[:], rps[:])
                    scl = gpool.tile([BT, G], F32, tag="scl", name=f"scl_{g}")
                    nc.vector.reciprocal(scl[:], rps[:])        # sig/d
                scl_tiles[g] = scl
                return rps16

            def tail_rows(g, rps16):
                # rps column -> ewT row P via SBUF->SBUF DMA scatter;
                # emitted ahead of B(prev) t2..5 so the tiny DMAs fire with
                # ~25us of slack (saves 64 thin PE transposes + row copies)
                for tg in range(G):
                    blkt = ewT_blocks[g][tg // NBLK]
                    sl = slice((tg % NBLK) * BT, (tg % NBLK + 1) * BT)
                    nc.sync.dma_start(blkt[P:P + 1, sl], rps16[:, tg:tg + 1])

            def phase_b_tile(g, tg, fine=False):
                scl = scl_tiles[g]
                i = g * G + tg
                blkt = ewT_blocks[g][tg // NBLK]
                sl = slice((tg % NBLK) * BT, (tg % NBLK + 1) * BT)
                ob = stpool.tile([BT, VS], F16, tag="ob", name=f"ob_{i}")
                rows = slice(i * BT, (i + 1) * BT)
                off = 0
                for jp, (w0, w1) in enumerate(VPAIRS):
                    w2 = w0 + w1
                    ps2 = pso.tile([BT, 1024], F32, tag="o", name=f"ps2_{i}_{jp}")
                    nc.tensor.matmul(
                        ps2[:, 0:w0], blkt[:, sl], m2aug[:, off:off + w0],
                        start=True, stop=True,
                    )
                    nc.tensor.matmul(
                        ps2[:, w0:w2], blkt[:, sl], m2aug[:, off + w0:off + w2],
                        start=True, stop=True,
                    )
                    # split each pair's drain ACT/DVE: halves run in parallel
                    # and the short quantum keeps the gate-tail chain nimble.
                    # a few jp3 first-halves go to DVE to balance engine load
                    if jp == 3 and tg in (1, 4):
                        nc.vector.tensor_scalar_mul(
                            ob[:, off:off + w0], ps2[:, 0:w0], scl[:, tg:tg + 1]
                        )
                    else:
                        nc.scalar.mul(
                            ob[:, off:off + w0], ps2[:, 0:w0], scl[:, tg:tg + 1]
                        )
                    nc.vector.tensor_scalar_mul(
                        ob[:, off + w0:off + w2], ps2[:, w0:w2], scl[:, tg:tg + 1]
                    )
                    off += w2
                    if jp == 1:
                        nc.sync.dma_start(d_out[rows, 0:off], ob[:, 0:off])
                nc.sync.dma_start(d_out[rows, 2048:VS], ob[:, 2048:VS])

            # ---- software-pipelined group loop --------------------------
            # B(g-1) tiles are interleaved between A(g) blocks so the PE
            # queue never stalls on phase-A consumers or the gate tail
            NG = NBT // G
            prev = None
            for g in range(NG):
                ewT_blocks[g] = []
                qall = gpool.tile([BT, G], F32, tag="qall", name=f"qall_{g}")
                dcall = gpool.tile([BT, 2 * G], F32, tag="dcall", name=f"dcall_{g}")
                phase_a_block(g, 0, qall, dcall)
                if prev is not None:
                    phase_b_tile(prev, 0)
                    phase_b_tile(prev, 1)
                phase_a_block(g, 1, qall, dcall)
                rps16 = tail_scalar(g, qall, dcall)
                tail_rows(g, rps16)
                if prev is not None:
                    for tg in range(2, 8):
                        phase_b_tile(prev, tg)
                prev = g
            for tg in range(G):
                phase_b_tile(prev, tg, fine=(tg >= 5))

    nc.compile()
    return nc


def _get_nc():
    if "nc" not in _cache:
        _cache["nc"] = _build()
    return _cache["nc"]


def make_in_maps(x, pattern_dict, attn_w, attn_b, self_w, self_b, out_w, out_b):
    x = np.ascontiguousarray(np.asarray(x, dtype=np.float32))
    pattern_dict = np.asarray(pattern_dict, dtype=np.float32)
    attn_w = np.asarray(attn_w, dtype=np.float32)
    attn_b = np.asarray(attn_b, dtype=np.float32)
    self_w = np.asarray(self_w, dtype=np.float32)
    self_b = np.asarray(self_b, dtype=np.float32)
    out_w = np.asarray(out_w, dtype=np.float32)
    out_b = np.asarray(out_b, dtype=np.float32)

    # host-side weight fusions (batch-independent preprocessing)
    m3h = pattern_dict @ (self_w - np.eye(D, dtype=np.float32))   # [P, D]
    m3aug = np.zeros((P + 1, DC), dtype=np.float32)
    m3aug[0:P, 0:D] = m3h
    m3aug[0:P, D] = 1.0
    m3aug[0:P, D + 1] = 2.0 * (m3h @ self_b)
    m2full = pattern_dict @ out_w                                  # [P, V]

    shared = {
        "xT": np.ascontiguousarray(x.T.astype(np.float16)),
        "attn_w": np.ascontiguousarray(attn_w.astype(np.float16)),
        "attn_b": np.ascontiguousarray(attn_b.reshape(P, 1)),
        "ident": np.eye(128, dtype=np.float16),
        "sb2": np.full((BT, 1), float(self_b @ self_b), dtype=np.float32),
        "m3aug": m3aug.astype(np.float16),
    }
    in_maps = []
    for c in range(NCORES):
        m = dict(shared)
        m2aug = np.empty((P + 1, VS), dtype=np.float32)
        m2aug[0:P, :] = m2full[:, c * VS:(c + 1) * VS]
        m2aug[P, :] = out_b[c * VS:(c + 1) * VS]
        m["m2aug"] = np.ascontiguousarray(m2aug.astype(np.float16))
        in_maps.append(m)
    return in_maps


def kernel(x, pattern_dict, attn_w, attn_b, self_w, self_b, out_w, out_b):
    in_maps = make_in_maps(
        x, pattern_dict, attn_w, attn_b, self_w, self_b, out_w, out_b
    )
    nc = _get_nc()
    res = run_bass_kernel_spmd(nc, in_maps, list(range(NCORES)))
    return np.concatenate(
        [res.results[c]["out"] for c in range(NCORES)], axis=1
    ).astype(np.float32)

